# revision 7
# baseline (speedup 1.0000x reference)
"""CTC-greedy-decode + embedding + LSTM + projection kernel for Trainium2.

v3: time-sharded LSTM with two interleaved time-chunk threads per core.

Phase A (batch-parallel): each core computes argmax + CTC compaction for its
32 batch columns -> compacted token stream [32, 32+T] (32 null pad in front).
Tokens are transposed to time-major chunks and exchanged with an AllToAll so
every core ends up with tokens for ALL 256 columns over its TWO time chunks.

Phase B (time-parallel): each core runs TWO independent LSTM recurrences
(chunks c*256+th*128, 32 warmup + 128 owned steps each, all 256 columns),
interleaved step-by-step so the tensor engine never idles waiting for the
elementwise tail (keeps the PE HAM clock-gate at full rate). x-projection is
folded into the gate PSUM accumulation via onehot @ E_fused matmuls; output
projection batched every 4 steps per thread. y is written time-sharded as
[V, 256*256] per core and reassembled on the host.
"""

import sys

sys.path.insert(0, "/opt/trn_rl_repo")

import numpy as np

import concourse.bass as bass
import concourse.tile as tile
from concourse import bacc, mybir
from concourse.bass import ds, ts
from concourse.bass_utils import run_bass_kernel_spmd
from concourse.masks import make_identity

F32 = mybir.dt.float32
BF16 = mybir.dt.bfloat16
I16 = mybir.dt.int16
ALU = mybir.AluOpType
ACTF = mybir.ActivationFunctionType
AXL = mybir.AxisListType

N_CORES = 8
T = 2048
B = 256
H = 256
V = 64
G4 = 4 * H  # 1024
BLANK = V - 1
BC = 32          # batch columns per core in phase A
NTH = 2          # interleaved time-chunk threads per core
K_WARM = 16      # LSTM warmup steps per time chunk
OWN = T // (N_CORES * NTH)   # owned steps per thread (128)
STEPS = OWN + K_WARM         # macro steps per thread (160)
NW_WARM = K_WARM // 8        # 4 warmup windows
NW_MAIN = OWN // 8           # 16 main windows
PADT = K_WARM + T            # padded token timeline (2080)

_cache = {}


def _emit(nc, tc, ctx, debug, d):
    x_d, y_d = d["x_d"], d["y_d"]
    emb_d, wih_d, whh_d = d["emb_d"], d["wih_d"], d["whh_d"]
    bih_d, bhh_d, wout_d, bout_d = d["bih_d"], d["bhh_d"], d["wout_d"], d["bout_d"]

    # ---------------- persistent tiles ----------------
    pp = ctx.enter_context(tc.tile_pool(name="persist", bufs=1))
    # torch gate chunk order: i(0,1) f(2,3) g(4,5) o(6,7); g folded with x2
    whhT = [[pp.tile([128, 128], BF16, name=f"whhT{k}{j}", tag=f"whhT{k}{j}")
             for j in range(8)] for k in range(2)]
    eT = [pp.tile([V, 128], BF16, name=f"eT{j}", tag=f"eT{j}") for j in range(8)]
    woutT = [pp.tile([128, V], BF16, name=f"woutT{k}", tag=f"woutT{k}")
             for k in range(2)]
    bout64 = pp.tile([V, 1], F32, tag="bout64")
    ident = pp.tile([128, 128], F32, tag="ident")
    iota_rev = pp.tile([128, BC * V], BF16, tag="iota_rev")
    iota_v = pp.tile([V, 1], F32, tag="iota_v")
    tokT = pp.tile([BC, T], F32, tag="tokT")        # raw argmax tokens [b, t]
    tokP = pp.tile([BC, PADT], F32, tag="tokP")     # padded compacted tokens
    hist = [pp.tile([128, 2 * 8 * B], BF16, name=f"hist{t_}", tag=f"hist{t_}")
            for t_ in range(NTH)]                   # h ring (k, slot8, b)
    c_st = [[pp.tile([128, 2 * B], BF16, name=f"c{t_}{i}", tag=f"c{t_}{i}")
             for i in range(2)] for t_ in range(NTH)]

    make_identity(nc, ident[:])
    nc.gpsimd.iota(iota_rev[:].rearrange("p (b v) -> p b v", v=V),
                   pattern=[[0, BC], [-1, V]], base=V - 1, channel_multiplier=0,
                   allow_small_or_imprecise_dtypes=True)
    nc.gpsimd.iota(iota_v[:], pattern=[[0, 1]], base=0, channel_multiplier=1,
                   allow_small_or_imprecise_dtypes=True)

    # DRAM bounce buffers for the token AllToAll (chunk-major flat layout).
    dramp = ctx.enter_context(tc.tile_pool(name="dram", bufs=1, space="DRAM"))
    a2a_in = dramp.tile([N_CORES, NTH * STEPS * BC], F32)
    a2a_out = dramp.tile([N_CORES, NTH * STEPS * BC], F32)

    # ---------------- setup: transpose weights, build E_fused.T ----------
    with tc.tile_pool(name="setup", bufs=2) as sp, \
         tc.tile_pool(name="setup_ps", bufs=2, space="PSUM") as spp:
        embT = [pp.tile([128, V], BF16, name=f"embT{k}", tag=f"embT{k}")
                for k in range(2)]
        bb = pp.tile([1, G4], F32, tag="bb")
        ones1 = pp.tile([1, V], F32, tag="ones1")

        for j in range(8):
            s_w = sp.tile([128, H], F32, tag="s_w")
            nc.sync.dma_start(s_w[:], whh_d.ap()[ts(j, 128), :])
            for k in range(2):
                pt = spp.tile([128, 128], F32, tag="pt")
                nc.tensor.transpose(pt[:], s_w[:, ts(k, 128)], ident[:])
                nc.vector.tensor_copy(out=whhT[k][j][:], in_=pt[:])
        s_e = sp.tile([V, H], F32, tag="s_e")
        nc.sync.dma_start(s_e[:], emb_d.ap()[:, :])
        for k in range(2):
            pt2 = spp.tile([128, V], F32, tag="pt2")
            nc.tensor.transpose(pt2[:], s_e[:, ts(k, 128)], ident[:V, :V])
            nc.vector.tensor_copy(out=embT[k][:], in_=pt2[:])
        s_bi = sp.tile([1, G4], F32, tag="s_bi")
        s_bh = sp.tile([1, G4], F32, tag="s_bh")
        nc.sync.dma_start(s_bi[:], bih_d.ap()[:, :])
        nc.sync.dma_start(s_bh[:], bhh_d.ap()[:, :])
        nc.vector.tensor_tensor(out=bb[:], in0=s_bi[:], in1=s_bh[:], op=ALU.add)
        nc.vector.memset(ones1[:], 1.0)
        for j in range(8):
            s_w = sp.tile([128, H], F32, tag="s_w")
            nc.sync.dma_start(s_w[:], wih_d.ap()[ts(j, 128), :])
            wT = [sp.tile([128, 128], BF16, name=f"s_wt{k}", tag=f"s_wt{k}")
                  for k in range(2)]
            for k in range(2):
                pt = spp.tile([128, 128], F32, tag="pt")
                nc.tensor.transpose(pt[:], s_w[:, ts(k, 128)], ident[:])
                nc.vector.tensor_copy(out=wT[k][:], in_=pt[:])
            pe = spp.tile([V, 128], F32, tag="pe")
            nc.tensor.matmul(pe[:], embT[0][:], wT[0][:], start=True, stop=False)
            nc.tensor.matmul(pe[:], embT[1][:], wT[1][:], start=False, stop=False)
            nc.tensor.matmul(pe[:], ones1[:], bb[:, ts(j, 128)],
                             start=False, stop=True)
            nc.vector.tensor_copy(out=eT[j][:], in_=pe[:])
        s_wo = sp.tile([V, H], F32, tag="s_e")
        nc.sync.dma_start(s_wo[:], wout_d.ap()[:, :])
        for k in range(2):
            pt2 = spp.tile([128, V], F32, tag="pt2")
            nc.tensor.transpose(pt2[:], s_wo[:, ts(k, 128)], ident[:V, :V])
            nc.vector.tensor_copy(out=woutT[k][:], in_=pt2[:])
        s_bo = sp.tile([1, V], F32, tag="s_bo")
        nc.sync.dma_start(s_bo[:], bout_d.ap()[:, :])
        nc.sync.dma_start(bout64[:, 0:1], s_bo[0:1, :])

    # ---------------- stage A: argmax ----------------
    xv = x_d.ap().rearrange("(n p) b v -> n p (b v)", p=128)
    with tc.tile_pool(name="argmax", bufs=3) as ag, \
         tc.tile_pool(name="argmax_ps", bufs=2, space="PSUM") as agp:
        for i in range(T // 128):
            xa = ag.tile([128, BC * V], F32, tag="xa")
            nc.sync.dma_start(xa[:], xv[i])
            xa3 = xa[:].rearrange("p (b v) -> p b v", v=V)
            mx = ag.tile([128, BC], F32, tag="mx")
            nc.vector.tensor_reduce(mx[:], xa3, axis=AXL.X, op=ALU.max)
            eq = ag.tile([128, BC * V], BF16, tag="eq")
            nc.vector.tensor_tensor(
                out=eq[:].rearrange("p (b v) -> p b v", v=V), in0=xa3,
                in1=mx[:].to_broadcast([128, BC, V]),
                op=ALU.is_ge)
            sel = ag.tile([128, BC * V], BF16, tag="sel")
            nc.vector.tensor_tensor(out=sel[:], in0=eq[:], in1=iota_rev[:],
                                    op=ALU.mult)
            am = ag.tile([128, BC], BF16, tag="am")
            nc.vector.tensor_reduce(am[:],
                                    sel[:].rearrange("p (b v) -> p b v", v=V),
                                    axis=AXL.X, op=ALU.max)
            tokf = ag.tile([128, BC], F32, tag="tokf")
            nc.vector.tensor_scalar(out=tokf[:], in0=am[:], scalar1=-1.0,
                                    scalar2=float(V - 1), op0=ALU.mult,
                                    op1=ALU.add)
            ptk = agp.tile([BC, 128], F32, tag="ptk")
            nc.tensor.transpose(ptk[:], tokf[:], ident[:])
            nc.vector.tensor_copy(out=tokT[:, ts(i, 128)], in_=ptk[:])

    # ---------------- stage B: CTC compaction ----------------
    with tc.tile_pool(name="ctc", bufs=1) as cp:
        nq = cp.tile([BC, T], F32, tag="nq")
        nc.vector.memset(nq[:, 0:1], 1.0)
        nc.vector.tensor_tensor(out=nq[:, 1:T], in0=tokT[:, 1:T],
                                in1=tokT[:, 0:T - 1], op=ALU.not_equal)
        nb = cp.tile([BC, T], F32, tag="nb")
        nc.vector.tensor_scalar(out=nb[:], in0=tokT[:], scalar1=float(BLANK),
                                scalar2=None, op0=ALU.not_equal)
        keep = cp.tile([BC, T], F32, tag="keep")
        nc.vector.tensor_tensor(out=keep[:], in0=nq[:], in1=nb[:], op=ALU.mult)
        ksc = cp.tile([BC, T], F32, tag="ksc")
        nc.vector.tensor_tensor_scan(out=ksc[:], data0=keep[:], data1=keep[:],
                                     initial=0.0, op0=ALU.add, op1=ALU.bypass)
        kidx = cp.tile([BC, T], F32, tag="kidx")
        nc.vector.tensor_tensor(out=kidx[:], in0=ksc[:], in1=keep[:],
                                op=ALU.mult)
        idx = cp.tile([BC, T], F32, tag="idx")
        nc.vector.tensor_scalar(out=idx[:], in0=kidx[:], scalar1=-1.0,
                                scalar2=None, op0=ALU.add)
        val = cp.tile([BC, T], BF16, tag="val")
        nc.vector.tensor_scalar(out=val[:], in0=tokT[:], scalar1=float(-BLANK),
                                scalar2=None, op0=ALU.add)
        tokc = cp.tile([BC, T], BF16, tag="tokc")
        n_half = T // 2
        for hf in range(2):
            m = cp.tile([BC, T], F32, tag="m")
            nc.vector.tensor_scalar(out=m[:], in0=idx[:], scalar1=float(n_half),
                                    scalar2=None,
                                    op0=(ALU.is_lt if hf == 0 else ALU.is_ge))
            a = cp.tile([BC, T], F32, tag="a")
            nc.vector.tensor_scalar(out=a[:], in0=idx[:],
                                    scalar1=float(1 - hf * n_half),
                                    scalar2=None, op0=ALU.add)
            am_ = cp.tile([BC, T], F32, tag="am_")
            nc.vector.tensor_tensor(out=am_[:], in0=a[:], in1=m[:], op=ALU.mult)
            i16 = cp.tile([BC, T], I16, tag="i16")
            nc.vector.tensor_scalar(out=i16[:], in0=am_[:], scalar1=-1.0,
                                    scalar2=None, op0=ALU.add)
            nc.gpsimd.local_scatter(
                out_ap=tokc[:, hf * n_half:(hf + 1) * n_half],
                data_ap=val[:], idxs_ap=i16[:], channels=BC,
                num_elems=n_half, num_idxs=T)
        nc.vector.memset(tokP[:, 0:K_WARM], float(V))  # null tokens (zero emb)
        nc.vector.tensor_scalar(out=tokP[:, K_WARM:PADT], in0=tokc[:],
                                scalar1=float(BLANK), scalar2=None, op0=ALU.add)
        if debug:
            nc.sync.dma_start(d["dtok_d"].ap()[:, :], tokP[:])

    # ------------- transpose token chunks + AllToAll ---------------------
    # chunk (c, th) = padded steps [c*256+th*128, +160) time-major [160, 32]
    a2a_in4 = a2a_in[:].rearrange("c (t s l) -> c t s l", t=NTH, l=BC)
    with tc.tile_pool(name="tp", bufs=3) as tp, \
         tc.tile_pool(name="tp_ps", bufs=3, space="PSUM") as tpp:
        for c8 in range(N_CORES):
            for th in range(NTH):
                base = c8 * 2 * OWN + th * OWN
                for q, rows in enumerate([128, STEPS - 128]):
                    pt = tpp.tile([128, BC], F32, tag="pt")
                    src = tokP[:, ds(base + q * 128, rows)]
                    nc.tensor.transpose(pt[:rows, :], src, ident[:BC, :BC])
                    sb = tp.tile([128, BC], F32, tag="sb")
                    nc.vector.tensor_copy(out=sb[:rows, :], in_=pt[:rows, :])
                    nc.sync.dma_start(a2a_in4[c8, th, ds(q * 128, rows), :],
                                      sb[:rows, :])
    nc.gpsimd.collective_compute(
        "AllToAll", ALU.bypass,
        replica_groups=[list(range(N_CORES))],
        ins=[a2a_in.opt()], outs=[a2a_out.opt()],
    )
    if debug:
        nc.gpsimd.dma_start(d["da2a_d"].ap()[:, :], a2a_out[:])

    # ---------------- phase B: time-sharded LSTM, 2 threads ----------------
    # a2a_out chunk p = peer p's 32 columns, [2, 160, 32] time-major.
    a2a_src = a2a_out[:].rearrange("c (t s l) -> t s c l", t=NTH, l=BC)

    mp = ctx.enter_context(tc.tile_pool(name="step", bufs=2))
    wpool = ctx.enter_context(tc.tile_pool(name="win", bufs=2))
    spool = ctx.enter_context(tc.tile_pool(name="sig", bufs=2))
    psg_p = ctx.enter_context(tc.tile_pool(name="psg", bufs=3, space="PSUM"))
    psy_p = ctx.enter_context(tc.tile_pool(name="psy", bufs=2, space="PSUM"))

    hist3 = []
    for t_ in range(NTH):
        nc.vector.memset(c_st[t_][0][:], 0.0)
        nc.vector.memset(hist[t_][:], 0.0)
        hist3.append(hist[t_][:].rearrange("p (k s b) -> p k s b", k=2, s=8))

    BURST_CHUNKS = [[4, 5, 0, 1], [2, 3, 6, 7]]  # tile0 = g,i; tile1 = f,o

    def step_burst_tile(th, oh, sl, tn):
        hp = [hist3[th][:, k, (sl - 1) % 8, :] for k in range(2)]
        chunks = BURST_CHUNKS[tn]
        ps = psg_p.tile([128, 4 * B], F32, tag="psT")
        for ix, ch in enumerate(chunks):
            # start=True clears has_written for the WHOLE 2KB bank, so
            # only the first matmul touching each bank may set it; the
            # second chunk's write lands on cleared bits -> overwrite.
            nc.tensor.matmul(ps[:, ts(ix, B)], eT[ch][:],
                             oh[:, ts(sl, B)], start=(ix % 2 == 0),
                             stop=False)
        for k in range(2):
            for ix, ch in enumerate(chunks):
                nc.tensor.matmul(ps[:, ts(ix, B)], whhT[k][ch][:],
                                 hp[k], start=False, stop=(k == 1))
        return ps

    def step_sig(th, psT):
        # psT[0] = [g, i] chunks, psT[1] = [f, o]
        tgh = spool.tile([128, 2 * B], BF16, tag=f"tgh{th}")
        nc.scalar.activation(tgh[:], psT[0][:, 0:2 * B], ACTF.Tanh)
        sgi = spool.tile([128, 2 * B], BF16, tag=f"sgi{th}")
        nc.scalar.activation(sgi[:], psT[0][:, 2 * B:4 * B], ACTF.Sigmoid)
        sfo = spool.tile([128, 4 * B], BF16, tag=f"sfo{th}")
        nc.scalar.activation(sfo[:], psT[1][:], ACTF.Sigmoid)
        return sgi, tgh, sfo

    def step_c(th, sl, sgi, tgh, sfo):
        c_prev = c_st[th][sl % 2]
        c_new = c_st[th][1 - sl % 2]
        # c = sig_f*c + sig_i*tanh(g)
        cf = mp.tile([128, 2 * B], BF16, tag=f"cf{th}")
        nc.vector.tensor_tensor(out=cf[:], in0=sfo[:, 0:2 * B],
                                in1=c_prev[:], op=ALU.mult)
        t2 = mp.tile([128, 2 * B], BF16, tag=f"t2{th}")
        nc.vector.tensor_tensor(out=t2[:], in0=tgh[:], in1=sgi[:],
                                op=ALU.mult)
        nc.vector.tensor_tensor(out=c_new[:], in0=cf[:], in1=t2[:], op=ALU.add)
        return c_new

    def step_tcs(th, c_new):
        tcs = mp.tile([128, 2 * B], BF16, tag=f"tcs{th}")
        nc.scalar.activation(tcs[:], c_new[:], ACTF.Tanh)
        return tcs

    def step_h(th, sl, sfo, tcs):
        # h = sig_o * tanh(c)
        nc.vector.tensor_tensor(
            out=hist3[th][:, :, sl, :],
            in0=tcs[:].rearrange("p (k b) -> p k b", k=2),
            in1=sfo[:, 2 * B:4 * B].rearrange("p (k b) -> p k b", k=2),
            op=ALU.mult)

    def yproj(th, q, wv):
        # 2 hist slots [2q, 2q+2) -> y, one PSUM bank, emitted 2 steps after
        # the slots were written so the DVE tail is off the critical path
        psy = psy_p.tile([V, 2 * B], F32, tag="psy")
        for k in range(2):
            rhs = hist[th][:, ds(k * 8 * B + q * 2 * B, 2 * B)]
            nc.tensor.matmul(psy[:], woutT[k][:], rhs,
                             start=(k == 0), stop=(k == 1))
        ysb = mp.tile([V, 2 * B], F32, tag="ysb")
        nc.vector.tensor_tensor(out=ysb[:], in0=psy[:],
                                in1=bout64[:, 0:1].to_broadcast([V, 2 * B]),
                                op=ALU.add)
        nc.sync.dma_start(
            y_d.ap()[:, ds(th * OWN * B + wv * 8 * B + q * 2 * B, 2 * B)],
            ysb[:])

    def build_oh(w):
        ohs = []
        for th in range(NTH):
            tok_row = wpool.tile([1, 8 * B], F32, tag=f"tok_row{th}")
            nc.sync.dma_start(tok_row[:], a2a_src[th, ds(w * 8, 8), :, :])
            tok_bc = wpool.tile([V, 8 * B], F32, tag=f"tok_bc{th}")
            nc.gpsimd.partition_broadcast(tok_bc[:], tok_row[:], channels=V)
            oh = wpool.tile([V, 8 * B], BF16, tag=f"oh{th}")
            nc.vector.tensor_scalar(out=oh[:], in0=tok_bc[:],
                                    scalar1=iota_v[:, 0:1], scalar2=None,
                                    op0=ALU.is_equal)
            ohs.append(oh)
        return ohs

    NW_TOT = NW_WARM + NW_MAIN
    ohs_cur = build_oh(0)
    for w in range(NW_TOT):
        warm = w < NW_WARM
        wv = w - NW_WARM
        ohs_next = build_oh(w + 1) if w + 1 < NW_TOT else None
        for sl in range(8):
            # interleave the two threads stage-by-stage so each in-order
            # engine queue sees work in data-ready order (no head-of-line
            # blocking of thread B behind thread A's dependent ops)
            ps = [[step_burst_tile(th, ohs_cur[th], sl, tn)
                   for tn in range(2)] for th in range(NTH)]
            sigs = [step_sig(th, ps[th]) for th in range(NTH)]
            cn = [step_c(th, sl, *sigs[th]) for th in range(NTH)]
            tcs = [step_tcs(th, cn[th]) for th in range(NTH)]
            for th in range(NTH):
                step_h(th, sl, sigs[th][2], tcs[th])
            if sl == 1 and wv >= 1:
                for th in range(NTH):
                    yproj(th, 3, wv - 1)
            if not warm and sl in (3, 5, 7):
                for th in range(NTH):
                    yproj(th, (sl - 3) // 2, wv)
        ohs_cur = ohs_next
    for th in range(NTH):
        yproj(th, 3, NW_MAIN - 1)


def _build(debug=False):
    from contextlib import ExitStack
    nc = bacc.Bacc("TRN2", target_bir_lowering=False, debug=False,
                   num_devices=N_CORES)
    d = {}
    d["x_d"] = nc.dram_tensor("x", [T, BC, V], F32, kind="ExternalInput")
    d["emb_d"] = nc.dram_tensor("emb", [V, H], F32, kind="ExternalInput")
    d["wih_d"] = nc.dram_tensor("W_ih", [G4, H], F32, kind="ExternalInput")
    d["whh_d"] = nc.dram_tensor("W_hh", [G4, H], F32, kind="ExternalInput")
    d["bih_d"] = nc.dram_tensor("b_ih", [1, G4], F32, kind="ExternalInput")
    d["bhh_d"] = nc.dram_tensor("b_hh", [1, G4], F32, kind="ExternalInput")
    d["wout_d"] = nc.dram_tensor("W_out", [V, H], F32, kind="ExternalInput")
    d["bout_d"] = nc.dram_tensor("b_out", [1, V], F32, kind="ExternalInput")
    d["y_d"] = nc.dram_tensor("y", [V, NTH * OWN * B], F32,
                              kind="ExternalOutput")
    if debug:
        d["dtok_d"] = nc.dram_tensor("dbg_tok", [BC, PADT], F32,
                                     kind="ExternalOutput")
        d["da2a_d"] = nc.dram_tensor("dbg_a2a", [N_CORES, NTH * STEPS * BC],
                                     F32, kind="ExternalOutput")
    with tile.TileContext(nc) as tc:
        with ExitStack() as ctx:
            _emit(nc, tc, ctx, debug, d)
    nc.compile()
    return nc


def _shard_inputs(x, emb, W_ih, W_hh, b_ih, b_hh, W_out, b_out):
    ins = []
    for c in range(N_CORES):
        ins.append({
            "x": np.ascontiguousarray(x[:, c * BC:(c + 1) * BC, :],
                                      dtype=np.float32),
            "emb": np.asarray(emb, np.float32),
            "W_ih": np.asarray(W_ih, np.float32),
            "W_hh": np.asarray(W_hh, np.float32),
            "b_ih": np.asarray(b_ih, np.float32).reshape(1, G4),
            "b_hh": np.asarray(b_hh, np.float32).reshape(1, G4),
            "W_out": np.asarray(W_out, np.float32),
            "b_out": np.asarray(b_out, np.float32).reshape(1, V),
        })
    return ins


def kernel(x, emb, W_ih, W_hh, b_ih, b_hh, W_out, b_out, _trace=False,
           _debug=False):
    x = np.asarray(x)
    key = _debug
    if key not in _cache:
        _cache[key] = _build(debug=_debug)
    nc = _cache[key]
    ins = _shard_inputs(x, emb, W_ih, W_hh, b_ih, b_hh, W_out, b_out)
    res = run_bass_kernel_spmd(nc, ins, core_ids=list(range(N_CORES)),
                               trace=_trace)
    y = np.concatenate(
        [res.results[c]["y"].reshape(V, NTH * OWN, B).transpose(1, 2, 0)
         for c in range(N_CORES)], axis=0)
    kernel.last_result = res
    return y
